# revision 22
# baseline (speedup 1.0000x reference)
"""Multi-head causal attention on 8 Trainium2 NeuronCores.

Sharding: core c -> (batch b = c//2, head-half hh = c%2).  Each core computes
q/k/v projections for its 8 heads (column-sharded wq/wk/wv), causal attention,
and a full-width partial output projection (row-sharded wo).  Host sums the
two partials per batch and adds the bias.

Device-side layout trick: scores are computed transposed (scoresT[j, i]) so
that the softmax-weighted sum over keys (ctx) is a plain matmul with v as the
stationary operand.  Ones-columns baked alongside v produce the softmax
denominator replicated across 64 partitions in the same PSUM tile as ctx.

Scheduling: a single interleaved program.  Projections for s-block sb+1 and
the output projection for i-block ib-1 are chopped into small generator
"filler units" issued between the score/ctx matmuls of attention block ib, so
the in-order PE queue always has independent work while the scalar engine
(exp) is the pacer.  Ctx PSUM accumulators are evicted to SBUF immediately
after the last ctx matmul (releasing the PSUM bank); the softmax
normalization (reciprocal rows, DMA partition-broadcast, multiplies) runs off
the critical path on the DVE/GpSimd/DMA engines.
"""

import numpy as np
import ml_dtypes

import concourse.bass as bass
import concourse.mybir as mybir
import concourse.tile as tile
from concourse import bacc
from concourse.bass_utils import run_bass_kernel_spmd

# Problem shape (hardcoded; kernel.py must be self-contained).
B, S, D, H = 4, 2048, 1024, 16
HD = D // H           # 64 head dim
NCORES = 8
EH = D // 2           # 512: per-core e-width (8 heads)
SB = 512              # s-block (free dim of most matmuls)
NSB = S // SB         # 4
NST = S // 128        # 16 s-tiles / j-tiles
NEG = EH // 128       # 4 e-groups of 128 partitions (head pairs)
NKG = D // 128        # 8 d-groups (contraction tiles)
VROW = 4 * 192        # v_ext row: 4x [v_odd(64) | ones(64) | v_even(64)]

F32 = mybir.dt.float32
BF16 = mybir.dt.bfloat16
MMDT = BF16
MMNP = ml_dtypes.bfloat16

TRACE = False
LAST_RESULT = None


def _build():
    nc = bacc.Bacc()

    xT_d = nc.dram_tensor("xt", [D, S], MMDT, kind="ExternalInput")
    wqT_d = nc.dram_tensor("wqt", [D, EH], MMDT, kind="ExternalInput")
    wkT_d = nc.dram_tensor("wkt", [D, EH], MMDT, kind="ExternalInput")
    wvT_d = nc.dram_tensor("wvt", [D, EH], MMDT, kind="ExternalInput")
    woT_d = nc.dram_tensor("wot", [EH, D], MMDT, kind="ExternalInput")
    masks_d = nc.dram_tensor("masks", [128, 128], MMDT, kind="ExternalInput")
    out_d = nc.dram_tensor("out", [S, D], F32, kind="ExternalOutput")
    scr_d = nc.dram_tensor("dscr", [4, NSB, 2, SB], F32)

    with tile.TileContext(nc) as tc:
        with (
            tc.tile_pool(name="persist", bufs=1) as persist,
            tc.tile_pool(name="wts", bufs=1) as wts,
            tc.tile_pool(name="xp", bufs=2) as xp,
            tc.tile_pool(name="expp", bufs=4) as expp,
            tc.tile_pool(name="crawp", bufs=2) as crawp,
            tc.tile_pool(name="rrp", bufs=2) as rrp,
            tc.tile_pool(name="bcp", bufs=2) as bcp,
            tc.tile_pool(name="otp", bufs=2) as otp,
            tc.tile_pool(name="sp", bufs=2, space="PSUM") as sp,
            tc.tile_pool(name="pcp", bufs=2, space="PSUM") as pcp,
            tc.tile_pool(name="accp", bufs=2, space="PSUM") as accp,
        ):
            qT = persist.tile([128, NEG, S], MMDT)      # [e-part, e-group, s]
            kT = persist.tile([128, NEG, S], MMDT)
            v_ext = persist.tile([128, NST, VROW], MMDT)  # [s-part, s-tile, row]
            ctxT = persist.tile([128, NEG, S], MMDT)

            masks_sb = wts.tile([128, 128], MMDT)
            w_q = wts.tile([128, NKG, EH], MMDT)
            w_k = wts.tile([128, NKG, EH], MMDT)
            w_v = wts.tile([128, NKG, EH], MMDT)
            woT_sb = wts.tile([128, NEG, D], MMDT)

            # ---- weight preloads (gpsimd dma queue); w_q first so the
            # first projection block can start as early as possible ----
            for kg in range(NKG):
                sl = slice(kg * 128, (kg + 1) * 128)
                nc.gpsimd.dma_start(out=w_q[:, kg, :], in_=wqT_d[sl, :])
            nc.gpsimd.dma_start(out=masks_sb, in_=masks_d[:, :])
            for kg in range(NKG):
                sl = slice(kg * 128, (kg + 1) * 128)
                nc.gpsimd.dma_start(out=w_k[:, kg, :], in_=wkT_d[sl, :])
                nc.gpsimd.dma_start(out=w_v[:, kg, :], in_=wvT_d[sl, :])
            for gg in range(NEG):
                nc.gpsimd.dma_start(
                    out=woT_sb[:, gg, :], in_=woT_d[gg * 128 : (gg + 1) * 128, :]
                )

            # shared ones block between each (even, odd) head pair; a single
            # strided memset covers all 64 blocks
            v_flat = v_ext[:].rearrange("p st (a w) -> p (st a) w", w=192)
            nc.vector.memset(v_flat[:, :, 64:128], 1.0)
            # f32 ones column block for the PE-broadcast normalize route
            onesb = wts.tile([128, 64], MMDT)
            nc.vector.memset(onesb, 1.0)

            xts = {}

            def dma_x(sb):
                t = xp.tile([128, NKG, SB], MMDT, tag="xts")
                for kg in range(NKG):
                    nc.sync.dma_start(
                        out=t[:, kg, :],
                        in_=xT_d[kg * 128 : (kg + 1) * 128, sb * SB : (sb + 1) * SB],
                    )
                xts[sb] = t

            # ---- filler generators: each yield = ~2 matmuls of issued work ----
            def gen_p1_qk(sb, w_sb, dst, mt):
                ssl = slice(sb * SB, (sb + 1) * SB)
                msl = slice(mt * 128, (mt + 1) * 128)
                x_t = xts[sb]
                ps = accp.tile([128, SB], F32, tag="acc")
                for kg in range(NKG):
                    nc.tensor.matmul(
                        out=ps,
                        lhsT=w_sb[:, kg, msl],
                        rhs=x_t[:, kg, :],
                        start=(kg == 0),
                        stop=(kg == NKG - 1),
                    )
                    if kg % 2 == 1:
                        yield
                nc.vector.tensor_copy(dst[:, mt, ssl], ps)
                yield

            def gen_p1_v(sb, st4):
                st = sb * 4 + st4
                x_t = xts[sb]
                xsl = slice(st4 * 128, (st4 + 1) * 128)
                ps = accp.tile([128, EH], F32, tag="acc")
                for kg in range(NKG):
                    nc.tensor.matmul(
                        out=ps,
                        lhsT=x_t[:, kg, xsl],
                        rhs=w_v[:, kg, :],
                        start=(kg == 0),
                        stop=(kg == NKG - 1),
                    )
                    if kg % 2 == 1:
                        yield
                # psum cols: head h at [h*64, h*64+64); dest pair p:
                # even head -> p*192+128, odd head -> p*192
                psr = ps[:].rearrange("p (a c) -> p a c", c=128)
                vst = v_ext[:, st, :].rearrange("p (a w) -> p a w", w=192)
                nc.vector.tensor_copy(vst[:, :, 128:192], psr[:, :, 0:64])
                nc.vector.tensor_copy(vst[:, :, 0:64], psr[:, :, 64:128])
                yield

            def gen_p3(ib, it, ob):
                itsl = slice(it * 128, (it + 1) * 128)
                osl = slice(ob * SB, (ob + 1) * SB)
                ps = accp.tile([128, SB], F32, tag="acc")
                for gg in range(NEG):
                    nc.tensor.matmul(
                        out=ps,
                        lhsT=ctxT[:, gg, itsl],
                        rhs=woT_sb[:, gg, osl],
                        start=(gg == 0),
                        stop=(gg == NEG - 1),
                    )
                    if gg % 2 == 1:
                        yield
                ot = otp.tile([128, SB], F32, tag="ot")
                nc.vector.tensor_copy(ot, ps)
                nc.sync.dma_start(out=out_d[itsl, osl], in_=ot)
                yield

            def p1_gens(sb, mts=range(NEG)):
                gens = []
                for mt in mts:
                    gens.append(gen_p1_qk(sb, w_q, qT, mt))
                    gens.append(gen_p1_qk(sb, w_k, kT, mt))
                return gens

            def p1_v_gens(sb):
                return [gen_p1_v(sb, st4) for st4 in range(4)]

            def p3_gens(ib):
                return [
                    gen_p3(ib, it, ob)
                    for it in range(4 * ib, 4 * ib + 4)
                    for ob in range(2)
                ]

            def run_all(gens):
                for g in gens:
                    for _ in g:
                        pass

            # ---- attention ----
            def scores(ib, pr, jt):
                r = jt - 4 * ib
                f0 = 128 * r if r > 0 else 0
                jsl = slice(jt * 128, (jt + 1) * 128)
                qsl = slice(ib * SB + f0, (ib + 1) * SB)
                ps_s = sp.tile([128, 2 * SB], F32, tag="s")
                nc.tensor.matmul(
                    out=ps_s[:, f0:SB],
                    lhsT=kT[0:64, pr, jsl],
                    rhs=qT[0:64, pr, qsl],
                    start=True,
                    stop=True,
                )
                nc.tensor.matmul(
                    out=ps_s[:, SB + f0 : 2 * SB],
                    lhsT=kT[64:128, pr, jsl],
                    rhs=qT[64:128, pr, qsl],
                    start=True,
                    stop=True,
                )
                return ps_s

            def softmax_ctx(ib, pr, jt, ps_s, ps_c0, ps_c1, njt, split=False):
                r = jt - 4 * ib
                f0 = 128 * r if r > 0 else 0
                expT = expp.tile([128, 2 * SB], MMDT, tag="exp")
                ps_v = ps_s[:].rearrange("p (t c) -> p t c", t=2)
                ex_v = expT[:].rearrange("p (t c) -> p t c", t=2)
                if not split:
                    nc.scalar.activation(
                        out=ex_v[:, :, f0:SB],
                        in_=ps_v[:, :, f0:SB],
                        func=mybir.ActivationFunctionType.Exp,
                        scale=1.0 / np.sqrt(HD),
                    )
                    if r >= 0:
                        nc.vector.tensor_mul(
                            ex_v[:, :, f0 : f0 + 128],
                            ex_v[:, :, f0 : f0 + 128],
                            masks_sb[:].unsqueeze(1).broadcast_to((128, 2, 128)),
                        )
                for t, ps_c in ((0, ps_c0), (1, ps_c1)):
                    if split:
                        # per-head exp halves the exp->ctx dependency latency
                        nc.scalar.activation(
                            out=ex_v[:, t, f0:SB],
                            in_=ps_v[:, t, f0:SB],
                            func=mybir.ActivationFunctionType.Exp,
                            scale=1.0 / np.sqrt(HD),
                        )
                        if r >= 0:
                            nc.vector.tensor_mul(
                                ex_v[:, t, f0 : f0 + 128],
                                ex_v[:, t, f0 : f0 + 128],
                                masks_sb[:, :],
                            )
                    coff = pr * 192 + (64 if t == 0 else 0)
                    nc.tensor.matmul(
                        out=ps_c[:, f0:SB],
                        lhsT=v_ext[:, jt, coff : coff + 128],
                        rhs=expT[:, t * SB + f0 : (t + 1) * SB],
                        start=(jt == 0),
                        stop=(jt == njt - 1),
                    )

            def attn(ib, fillers, n_units):
                njt = 4 * (ib + 1)
                isl = slice(ib * SB, (ib + 1) * SB)
                slots = 4 * (njt + 1)
                rate = n_units / slots if slots else 0.0
                queue = list(fillers)
                state = {"acc": 0.0}

                def drive():
                    state["acc"] += rate
                    while state["acc"] >= 1.0 and queue:
                        try:
                            next(queue[0])
                            state["acc"] -= 1.0
                        except StopIteration:
                            queue.pop(0)

                for pr in range(4):
                    ps_c0 = pcp.tile([128, SB], F32, tag="pc")
                    ps_c1 = pcp.tile([128, SB], F32, tag="pc")
                    prev = None
                    for jt in range(njt):
                        ps_prev = prev
                        prev = (jt, scores(ib, pr, jt))
                        drive()
                        if ps_prev is not None:
                            softmax_ctx(ib, pr, *ps_prev, ps_c0, ps_c1, njt,
                                        split=(ib < NSB - 1))
                    drive()
                    softmax_ctx(ib, pr, *prev, ps_c0, ps_c1, njt,
                                split=(ib < NSB - 1))

                    craw = crawp.tile([128, 2, SB], F32, tag="craw")

                    if ib == NSB - 1 and pr == 3:
                        # Last normalize gates the final output projection:
                        # avoid the DRAM round trip by broadcasting across
                        # partitions with 1-row PE matmuls instead.  Read the
                        # denominator rows straight from PSUM so the chain
                        # does not wait on the full eviction copies.
                        rowb = rrp.tile([128, 2, SB], MMDT, tag="rowb")
                        rr = rrp.tile([128, 2, SB], F32, tag="rr")
                        nc.vector.reciprocal_approx_fast(
                            rr[0:1, 0, :], ps_c0[0:1, :]
                        )
                        nc.vector.tensor_copy(rowb[0:1, 0, :], rr[0:1, 0, :])
                        nc.vector.tensor_copy(rowb[64:65, 1, :], ps_c1[64:65, :])
                        nc.vector.tensor_copy(craw[:, 1, :], ps_c1)
                        nc.vector.tensor_copy(craw[:, 0, :], ps_c0)
                        ps_b = accp.tile([128, SB], F32, tag="acc")
                        nc.tensor.matmul(
                            out=ps_b[64:128, :],
                            lhsT=onesb[0:1, :],
                            rhs=rowb[0:1, 0, :],
                            start=True,
                            stop=True,
                        )
                        ps_o = accp.tile([128, SB], F32, tag="acc")
                        nc.tensor.matmul(
                            out=ps_o[0:64, :],
                            lhsT=onesb[64:65, :],
                            rhs=rowb[64:65, 1, :],
                            start=True,
                            stop=True,
                        )
                        nc.vector.reciprocal_approx_fast(
                            rr[0:64, 1, :], ps_o[0:64, :]
                        )
                        nc.vector.tensor_mul(
                            ctxT[64:128, pr, isl], craw[64:128, 0, :],
                            ps_b[64:128, :],
                        )
                        nc.vector.tensor_mul(
                            ctxT[0:64, pr, isl], craw[0:64, 1, :], rr[0:64, 1, :]
                        )
                        continue

                    # Evict ctx+denominator PSUM to SBUF promptly (frees the
                    # pc bufs for the next pr), then normalize off-path.
                    nc.vector.tensor_copy(craw[:, 1, :], ps_c1)
                    nc.vector.tensor_copy(craw[:, 0, :], ps_c0)
                    # Odd-head raw denominator row (DMA cannot read PSUM).
                    nc.sync.dma_start(
                        out=scr_d[pr, ib, 1, :], in_=craw[64:65, 1, :]
                    )

                    # Even head: reciprocal of the partition-0 denominator row
                    # (custom-DVE ops misbehave on partition ranges that do
                    # not start at 0), broadcast the reciprocal to partitions
                    # 64:128 via a DRAM round trip.
                    rr = rrp.tile([128, 2, SB], F32, tag="rr")
                    nc.vector.reciprocal_approx_fast(
                        rr[0:1, 0, :], craw[0:1, 0, :]
                    )
                    nc.sync.dma_start(out=scr_d[pr, ib, 0, :], in_=rr[0:1, 0, :])
                    bc = bcp.tile([128, SB], F32, tag="bc")
                    se = scr_d[pr, ib, 0, :]
                    nc.sync.dma_start(
                        out=bc[64:128, :],
                        in_=bass.AP(
                            tensor=se.tensor, offset=se.offset,
                            ap=[[0, 64], [1, SB]],
                        ),
                    )
                    # Odd head: broadcast the raw denominator to partitions
                    # 0:64 and reciprocal there (partition offset 0).
                    so = scr_d[pr, ib, 1, :]
                    nc.sync.dma_start(
                        out=bc[0:64, :],
                        in_=bass.AP(
                            tensor=so.tensor, offset=so.offset,
                            ap=[[0, 64], [1, SB]],
                        ),
                    )
                    nc.vector.reciprocal_approx_fast(
                        rr[0:64, 1, :], bc[0:64, :]
                    )
                    # normalize: even head ctx rows 64:128 of plane 0, odd
                    # head rows 0:64 of plane 1
                    nc.vector.tensor_mul(
                        ctxT[64:128, pr, isl], craw[64:128, 0, :], bc[64:128, :]
                    )
                    nc.vector.tensor_mul(
                        ctxT[0:64, pr, isl], craw[0:64, 1, :], rr[0:64, 1, :]
                    )

                # drain leftover fillers
                run_all(queue)

            # ---- main schedule ----
            dma_x(0)
            # Dummy matmuls with no data dependencies: they execute during
            # the DMA head and ramp the PE clock out of its low power state
            # before the real work arrives.  Results are never read.
            dum = wts.tile([128, 128], MMDT)
            nc.vector.memset(dum[0:1, :], 1.0)
            for _ in range(24):
                ps_dum = accp.tile([128, 128], F32, tag="acc")
                nc.tensor.matmul(
                    out=ps_dum,
                    lhsT=dum[0:1, :],
                    rhs=dum[0:1, 0:128],
                    start=True,
                    stop=True,
                )
            # minimal prefix of projections so attention(0, pr=0) can start
            run_all(p1_gens(0, mts=[0]) + p1_v_gens(0))
            dma_x(1)
            for sbi in range(NSB):
                fillers = []
                n_units = 0
                if sbi == 0:
                    fillers += p1_gens(0, mts=[1, 2, 3])
                    n_units += 30
                if sbi == NSB - 1:
                    # k(sb=3) blocks for head pairs 1..3, deferred here from
                    # attn(2): attention only reads them from j-tile 12 of
                    # the matching pr, so they fill this scalar-paced block.
                    fillers += [
                        gen_p1_qk(sbi, w_k, kT, mt) for mt in (1, 2, 3)
                    ]
                    n_units += 15
                if sbi < NSB - 1:
                    if sbi > 0:
                        dma_x(sbi + 1)
                    if sbi == NSB - 2:
                        fillers += [
                            gen_p1_qk(sbi + 1, w_q, qT, mt) for mt in range(NEG)
                        ]
                        fillers += [gen_p1_qk(sbi + 1, w_k, kT, 0)]
                        fillers += p1_v_gens(sbi + 1)
                        n_units += 45
                    else:
                        fillers += p1_gens(sbi + 1) + p1_v_gens(sbi + 1)
                        n_units += 60
                if sbi >= 1:
                    fillers += p3_gens(sbi - 1)
                    n_units += 24
                attn(sbi, fillers, n_units)
            run_all(p3_gens(NSB - 1))

    nc.finalize()
    return nc


_NC = None


def _get_nc():
    global _NC
    if _NC is None:
        _NC = _build()
    return _NC


def kernel(x, wq, wk, wv, wo, wo_b):
    global LAST_RESULT
    x = np.ascontiguousarray(np.asarray(x, dtype=np.float32))
    wq = np.asarray(wq, dtype=np.float32)
    wk = np.asarray(wk, dtype=np.float32)
    wv = np.asarray(wv, dtype=np.float32)
    wo = np.asarray(wo, dtype=np.float32)
    wo_b = np.asarray(wo_b, dtype=np.float32)

    pp, ff = np.ogrid[0:128, 0:128]
    masks = (pp <= ff).astype(np.float32)

    in_maps = []
    for c in range(NCORES):
        b, hh = c // 2, c % 2
        es = slice(hh * EH, (hh + 1) * EH)
        in_maps.append(
            {
                "xt": np.ascontiguousarray(x[b].T.astype(MMNP)),
                "wqt": np.ascontiguousarray(wq[es, :].T.astype(MMNP)),
                "wkt": np.ascontiguousarray(wk[es, :].T.astype(MMNP)),
                "wvt": np.ascontiguousarray(wv[es, :].T.astype(MMNP)),
                "wot": np.ascontiguousarray(
                    wo[:, es].T.astype(MMNP)
                    .reshape(4, 2, 64, D)[:, ::-1]
                    .reshape(EH, D)
                ),
                "masks": masks.astype(MMNP),
            }
        )

    nc = _get_nc()
    res = run_bass_kernel_spmd(nc, in_maps, list(range(NCORES)), trace=TRACE)
    LAST_RESULT = res

    out = np.empty((B, S, D), np.float32)
    for b in range(B):
        out[b] = res.results[2 * b]["out"] + res.results[2 * b + 1]["out"]
    out += wo_b[None, None, :]
    return out


# revision 26
# speedup vs baseline: 1.0171x; 1.0171x over previous
"""Multi-head causal attention on 8 Trainium2 NeuronCores.

Sharding: core c -> (batch b = c//2, head-half hh = c%2).  Each core computes
q/k/v projections for its 8 heads (column-sharded wq/wk/wv), causal attention,
and a full-width partial output projection (row-sharded wo).  Host sums the
two partials per batch and adds the bias.

Device-side layout trick: scores are computed transposed (scoresT[j, i]) so
that the softmax-weighted sum over keys (ctx) is a plain matmul with v as the
stationary operand.  Ones-columns baked alongside v produce the softmax
denominator replicated across 64 partitions in the same PSUM tile as ctx.

Scheduling: a single interleaved program.  Projections for s-block sb+1 and
the output projection for i-block ib-1 are chopped into small generator
"filler units" issued between the score/ctx matmuls of attention block ib, so
the in-order PE queue always has independent work while the scalar engine
(exp) is the pacer.  Ctx PSUM accumulators are evicted to SBUF immediately
after the last ctx matmul (releasing the PSUM bank); the softmax
normalization (reciprocal rows, DMA partition-broadcast, multiplies) runs off
the critical path on the DVE/GpSimd/DMA engines.
"""

import numpy as np
import ml_dtypes

import concourse.bass as bass
import concourse.mybir as mybir
import concourse.tile as tile
from concourse import bacc
from concourse.bass_utils import run_bass_kernel_spmd

# Problem shape (hardcoded; kernel.py must be self-contained).
B, S, D, H = 4, 2048, 1024, 16
HD = D // H           # 64 head dim
NCORES = 8
EH = D // 2           # 512: per-core e-width (8 heads)
SB = 512              # s-block (free dim of most matmuls)
NSB = S // SB         # 4
NST = S // 128        # 16 s-tiles / j-tiles
NEG = EH // 128       # 4 e-groups of 128 partitions (head pairs)
NKG = D // 128        # 8 d-groups (contraction tiles)
VROW = 4 * 192        # v_ext row: 4x [v_odd(64) | ones(64) | v_even(64)]

F32 = mybir.dt.float32
BF16 = mybir.dt.bfloat16
MMDT = BF16
MMNP = ml_dtypes.bfloat16

TRACE = False
LAST_RESULT = None


def _build():
    nc = bacc.Bacc()

    xT_d = nc.dram_tensor("xt", [D, S], MMDT, kind="ExternalInput")
    wqT_d = nc.dram_tensor("wqt", [D, EH], MMDT, kind="ExternalInput")
    wkT_d = nc.dram_tensor("wkt", [D, EH], MMDT, kind="ExternalInput")
    wvT_d = nc.dram_tensor("wvt", [D, EH], MMDT, kind="ExternalInput")
    woT_d = nc.dram_tensor("wot", [EH, D], MMDT, kind="ExternalInput")
    masks_d = nc.dram_tensor("masks", [128, 128], MMDT, kind="ExternalInput")
    out_d = nc.dram_tensor("out", [S, D], F32, kind="ExternalOutput")
    scr_d = nc.dram_tensor("dscr", [4, NSB, 2, SB], F32)

    with tile.TileContext(nc) as tc:
        with (
            tc.tile_pool(name="persist", bufs=1) as persist,
            tc.tile_pool(name="wts", bufs=1) as wts,
            tc.tile_pool(name="xp", bufs=2) as xp,
            tc.tile_pool(name="expp", bufs=4) as expp,
            tc.tile_pool(name="crawp", bufs=2) as crawp,
            tc.tile_pool(name="rrp", bufs=2) as rrp,
            tc.tile_pool(name="bcp", bufs=2) as bcp,
            tc.tile_pool(name="otp", bufs=2) as otp,
            tc.tile_pool(name="sp", bufs=2, space="PSUM") as sp,
            tc.tile_pool(name="pcp", bufs=2, space="PSUM") as pcp,
            tc.tile_pool(name="accp", bufs=2, space="PSUM") as accp,
        ):
            qT = persist.tile([128, NEG, S], MMDT)      # [e-part, e-group, s]
            kT = persist.tile([128, NEG, S], MMDT)
            v_ext = persist.tile([128, NST, VROW], MMDT)  # [s-part, s-tile, row]
            ctxT = persist.tile([128, NEG, S], MMDT)

            masks_sb = wts.tile([128, 128], MMDT)
            w_q = wts.tile([128, NKG, EH], MMDT)
            w_k = wts.tile([128, NKG, EH], MMDT)
            w_v = wts.tile([128, NKG, EH], MMDT)
            woT_sb = wts.tile([128, NEG, D], MMDT)

            # ---- weight preloads split across the two DMA-issuing queues
            # (gpsimd: w_q then w_v; sync gets x(sb=0) first, then w_k) so
            # the first q/k/v projection blocks are fed as early as possible
            for kg in range(NKG):
                sl = slice(kg * 128, (kg + 1) * 128)
                nc.gpsimd.dma_start(out=w_q[:, kg, :], in_=wqT_d[sl, :])
            for kg in range(NKG):
                sl = slice(kg * 128, (kg + 1) * 128)
                nc.gpsimd.dma_start(out=w_v[:, kg, :], in_=wvT_d[sl, :])
            for gg in range(NEG):
                nc.gpsimd.dma_start(
                    out=woT_sb[:, gg, :], in_=woT_d[gg * 128 : (gg + 1) * 128, :]
                )

            # dummy-warmup source row first: it unblocks the PE ramp matmuls
            dum0 = wts.tile([128, SB], MMDT)
            nc.vector.memset(dum0[0:1, :], 1.0)
            # shared ones block between each (even, odd) head pair; a single
            # strided memset covers all 64 blocks
            v_flat = v_ext[:].rearrange("p st (a w) -> p (st a) w", w=192)
            nc.vector.memset(v_flat[:, :, 64:128], 1.0)
            # f32 ones column block for the PE-broadcast normalize route
            onesb = wts.tile([128, 64], MMDT)
            nc.vector.memset(onesb, 1.0)

            xts = {}

            def dma_x(sb):
                t = xp.tile([128, NKG, SB], MMDT, tag="xts")
                for kg in range(NKG):
                    nc.sync.dma_start(
                        out=t[:, kg, :],
                        in_=xT_d[kg * 128 : (kg + 1) * 128, sb * SB : (sb + 1) * SB],
                    )
                xts[sb] = t

            # ---- filler generators: each yield = ~2 matmuls of issued work ----
            def gen_p1_qk(sb, w_sb, dst, mt):
                ssl = slice(sb * SB, (sb + 1) * SB)
                msl = slice(mt * 128, (mt + 1) * 128)
                x_t = xts[sb]
                ps = accp.tile([128, SB], F32, tag="acc")
                for kg in range(NKG):
                    nc.tensor.matmul(
                        out=ps,
                        lhsT=w_sb[:, kg, msl],
                        rhs=x_t[:, kg, :],
                        start=(kg == 0),
                        stop=(kg == NKG - 1),
                    )
                    if kg % 2 == 1:
                        yield
                nc.vector.tensor_copy(dst[:, mt, ssl], ps)
                yield

            def gen_p1_v(sb, st4):
                st = sb * 4 + st4
                x_t = xts[sb]
                xsl = slice(st4 * 128, (st4 + 1) * 128)
                ps = accp.tile([128, EH], F32, tag="acc")
                for kg in range(NKG):
                    nc.tensor.matmul(
                        out=ps,
                        lhsT=x_t[:, kg, xsl],
                        rhs=w_v[:, kg, :],
                        start=(kg == 0),
                        stop=(kg == NKG - 1),
                    )
                    if kg % 2 == 1:
                        yield
                # psum cols: head h at [h*64, h*64+64); dest pair p:
                # even head -> p*192+128, odd head -> p*192
                psr = ps[:].rearrange("p (a c) -> p a c", c=128)
                vst = v_ext[:, st, :].rearrange("p (a w) -> p a w", w=192)
                nc.vector.tensor_copy(vst[:, :, 128:192], psr[:, :, 0:64])
                nc.vector.tensor_copy(vst[:, :, 0:64], psr[:, :, 64:128])
                yield

            def gen_p3(ib, it, ob):
                itsl = slice(it * 128, (it + 1) * 128)
                osl = slice(ob * SB, (ob + 1) * SB)
                ps = accp.tile([128, SB], F32, tag="acc")
                for gg in range(NEG):
                    nc.tensor.matmul(
                        out=ps,
                        lhsT=ctxT[:, gg, itsl],
                        rhs=woT_sb[:, gg, osl],
                        start=(gg == 0),
                        stop=(gg == NEG - 1),
                    )
                    if gg % 2 == 1:
                        yield
                ot = otp.tile([128, SB], F32, tag="ot")
                nc.vector.tensor_copy(ot, ps)
                nc.sync.dma_start(out=out_d[itsl, osl], in_=ot)
                yield

            def p1_gens(sb, mts=range(NEG)):
                gens = []
                for mt in mts:
                    gens.append(gen_p1_qk(sb, w_q, qT, mt))
                    gens.append(gen_p1_qk(sb, w_k, kT, mt))
                return gens

            def p1_v_gens(sb):
                return [gen_p1_v(sb, st4) for st4 in range(4)]

            def p3_gens(ib):
                return [
                    gen_p3(ib, it, ob)
                    for it in range(4 * ib, 4 * ib + 4)
                    for ob in range(2)
                ]

            def run_all(gens):
                for g in gens:
                    for _ in g:
                        pass

            # ---- attention ----
            def scores(ib, pr, jt):
                r = jt - 4 * ib
                f0 = 128 * r if r > 0 else 0
                jsl = slice(jt * 128, (jt + 1) * 128)
                qsl = slice(ib * SB + f0, (ib + 1) * SB)
                ps_s = sp.tile([128, 2 * SB], F32, tag="s")
                nc.tensor.matmul(
                    out=ps_s[:, f0:SB],
                    lhsT=kT[0:64, pr, jsl],
                    rhs=qT[0:64, pr, qsl],
                    start=True,
                    stop=True,
                )
                nc.tensor.matmul(
                    out=ps_s[:, SB + f0 : 2 * SB],
                    lhsT=kT[64:128, pr, jsl],
                    rhs=qT[64:128, pr, qsl],
                    start=True,
                    stop=True,
                )
                return ps_s

            def softmax_ctx(ib, pr, jt, ps_s, ps_c0, ps_c1, njt, split=False):
                r = jt - 4 * ib
                f0 = 128 * r if r > 0 else 0
                expT = expp.tile([128, 2 * SB], MMDT, tag="exp")
                ps_v = ps_s[:].rearrange("p (t c) -> p t c", t=2)
                ex_v = expT[:].rearrange("p (t c) -> p t c", t=2)
                if not split:
                    nc.scalar.activation(
                        out=ex_v[:, :, f0:SB],
                        in_=ps_v[:, :, f0:SB],
                        func=mybir.ActivationFunctionType.Exp,
                        scale=1.0 / np.sqrt(HD),
                    )
                    if r >= 0:
                        nc.vector.tensor_mul(
                            ex_v[:, :, f0 : f0 + 128],
                            ex_v[:, :, f0 : f0 + 128],
                            masks_sb[:].unsqueeze(1).broadcast_to((128, 2, 128)),
                        )
                for t, ps_c in ((0, ps_c0), (1, ps_c1)):
                    if split:
                        # per-head exp halves the exp->ctx dependency latency
                        nc.scalar.activation(
                            out=ex_v[:, t, f0:SB],
                            in_=ps_v[:, t, f0:SB],
                            func=mybir.ActivationFunctionType.Exp,
                            scale=1.0 / np.sqrt(HD),
                        )
                        if r >= 0:
                            nc.vector.tensor_mul(
                                ex_v[:, t, f0 : f0 + 128],
                                ex_v[:, t, f0 : f0 + 128],
                                masks_sb[:, :],
                            )
                    coff = pr * 192 + (64 if t == 0 else 0)
                    nc.tensor.matmul(
                        out=ps_c[:, f0:SB],
                        lhsT=v_ext[:, jt, coff : coff + 128],
                        rhs=expT[:, t * SB + f0 : (t + 1) * SB],
                        start=(jt == 0),
                        stop=(jt == njt - 1),
                    )

            def attn(ib, fillers, n_units):
                njt = 4 * (ib + 1)
                isl = slice(ib * SB, (ib + 1) * SB)
                slots = 4 * (njt + 1)
                rate = n_units / slots if slots else 0.0
                queue = list(fillers)
                state = {"acc": 0.0}

                def drive():
                    state["acc"] += rate
                    while state["acc"] >= 1.0 and queue:
                        try:
                            next(queue[0])
                            state["acc"] -= 1.0
                        except StopIteration:
                            queue.pop(0)

                for pr in range(4):
                    ps_c0 = pcp.tile([128, SB], F32, tag="pc")
                    ps_c1 = pcp.tile([128, SB], F32, tag="pc")
                    prev = None
                    for jt in range(njt):
                        ps_prev = prev
                        prev = (jt, scores(ib, pr, jt))
                        drive()
                        if ps_prev is not None:
                            softmax_ctx(ib, pr, *ps_prev, ps_c0, ps_c1, njt,
                                        split=(ib < NSB - 1))
                    drive()
                    softmax_ctx(ib, pr, *prev, ps_c0, ps_c1, njt,
                                split=(ib < NSB - 1))

                    craw = crawp.tile([128, 2, SB], F32, tag="craw")

                    if ib == NSB - 1 and pr == 3:
                        # Last normalize gates the final output projection:
                        # avoid the DRAM round trip by broadcasting across
                        # partitions with 1-row PE matmuls instead.  Read the
                        # denominator rows straight from PSUM so the chain
                        # does not wait on the full eviction copies.
                        rowb = rrp.tile([128, 2, SB], MMDT, tag="rowb")
                        rr = rrp.tile([128, 2, SB], F32, tag="rr")
                        nc.vector.reciprocal_approx_fast(
                            rr[0:1, 0, :], ps_c0[0:1, :]
                        )
                        nc.vector.tensor_copy(rowb[0:1, 0, :], rr[0:1, 0, :])
                        nc.vector.tensor_copy(rowb[64:65, 1, :], ps_c1[64:65, :])
                        nc.vector.tensor_copy(craw[:, 1, :], ps_c1)
                        nc.vector.tensor_copy(craw[:, 0, :], ps_c0)
                        ps_b = accp.tile([128, SB], F32, tag="acc")
                        nc.tensor.matmul(
                            out=ps_b[64:128, :],
                            lhsT=onesb[0:1, :],
                            rhs=rowb[0:1, 0, :],
                            start=True,
                            stop=True,
                        )
                        ps_o = accp.tile([128, SB], F32, tag="acc")
                        nc.tensor.matmul(
                            out=ps_o[0:64, :],
                            lhsT=onesb[64:65, :],
                            rhs=rowb[64:65, 1, :],
                            start=True,
                            stop=True,
                        )
                        nc.vector.reciprocal_approx_fast(
                            rr[0:64, 1, :], ps_o[0:64, :]
                        )
                        nc.vector.tensor_mul(
                            ctxT[64:128, pr, isl], craw[64:128, 0, :],
                            ps_b[64:128, :],
                        )
                        nc.vector.tensor_mul(
                            ctxT[0:64, pr, isl], craw[0:64, 1, :], rr[0:64, 1, :]
                        )
                        continue

                    # Evict ctx+denominator PSUM to SBUF promptly (frees the
                    # pc bufs for the next pr), then normalize off-path.
                    nc.vector.tensor_copy(craw[:, 1, :], ps_c1)
                    nc.vector.tensor_copy(craw[:, 0, :], ps_c0)
                    # Odd-head raw denominator row (DMA cannot read PSUM).
                    nc.sync.dma_start(
                        out=scr_d[pr, ib, 1, :], in_=craw[64:65, 1, :]
                    )

                    # Even head: reciprocal of the partition-0 denominator row
                    # (custom-DVE ops misbehave on partition ranges that do
                    # not start at 0), broadcast the reciprocal to partitions
                    # 64:128 via a DRAM round trip.
                    rr = rrp.tile([128, 2, SB], F32, tag="rr")
                    nc.vector.reciprocal_approx_fast(
                        rr[0:1, 0, :], craw[0:1, 0, :]
                    )
                    nc.sync.dma_start(out=scr_d[pr, ib, 0, :], in_=rr[0:1, 0, :])
                    bc = bcp.tile([128, SB], F32, tag="bc")
                    se = scr_d[pr, ib, 0, :]
                    nc.sync.dma_start(
                        out=bc[64:128, :],
                        in_=bass.AP(
                            tensor=se.tensor, offset=se.offset,
                            ap=[[0, 64], [1, SB]],
                        ),
                    )
                    # Odd head: broadcast the raw denominator to partitions
                    # 0:64 and reciprocal there (partition offset 0).
                    so = scr_d[pr, ib, 1, :]
                    nc.sync.dma_start(
                        out=bc[0:64, :],
                        in_=bass.AP(
                            tensor=so.tensor, offset=so.offset,
                            ap=[[0, 64], [1, SB]],
                        ),
                    )
                    nc.vector.reciprocal_approx_fast(
                        rr[0:64, 1, :], bc[0:64, :]
                    )
                    # normalize: even head ctx rows 64:128 of plane 0, odd
                    # head rows 0:64 of plane 1
                    nc.vector.tensor_mul(
                        ctxT[64:128, pr, isl], craw[64:128, 0, :], bc[64:128, :]
                    )
                    nc.vector.tensor_mul(
                        ctxT[0:64, pr, isl], craw[0:64, 1, :], rr[0:64, 1, :]
                    )

                # drain leftover fillers
                run_all(queue)

            # ---- main schedule ----
            dma_x(0)
            for kg in range(NKG):
                sl = slice(kg * 128, (kg + 1) * 128)
                nc.sync.dma_start(out=w_k[:, kg, :], in_=wkT_d[sl, :])
            nc.sync.dma_start(out=masks_sb, in_=masks_d[:, :])
            # Dummy matmuls with no data dependencies: they execute during
            # the DMA head and ramp the PE clock out of its low power state
            # before the real work arrives.  Results are never read.
            for i in range(32):
                ps_dum = accp.tile([128, SB], F32, tag="acc")
                nc.tensor.matmul(
                    out=ps_dum[:, 0 : (128 if i < 20 else SB)],
                    lhsT=dum0[0:1, 0:128],
                    rhs=dum0[0:1, 0 : (128 if i < 20 else SB)],
                    start=True,
                    stop=True,
                )
            # minimal prefix of projections so attention(0, pr=0) can start
            run_all(p1_gens(0, mts=[0]) + p1_v_gens(0))
            dma_x(1)
            for sbi in range(NSB):
                fillers = []
                n_units = 0
                if sbi == 0:
                    fillers += p1_gens(0, mts=[1, 2, 3])
                    n_units += 30
                if sbi == NSB - 1:
                    # k(sb=3) blocks for head pairs 1..3, deferred here from
                    # attn(2): attention only reads them from j-tile 12 of
                    # the matching pr, so they fill this scalar-paced block.
                    fillers += [
                        gen_p1_qk(sbi, w_k, kT, mt) for mt in (1, 2, 3)
                    ]
                    n_units += 15
                if sbi < NSB - 1:
                    if sbi > 0:
                        dma_x(sbi + 1)
                    if sbi == NSB - 2:
                        fillers += [
                            gen_p1_qk(sbi + 1, w_q, qT, mt) for mt in range(NEG)
                        ]
                        fillers += [gen_p1_qk(sbi + 1, w_k, kT, 0)]
                        fillers += p1_v_gens(sbi + 1)
                        n_units += 45
                    else:
                        fillers += p1_gens(sbi + 1) + p1_v_gens(sbi + 1)
                        n_units += 60
                if sbi >= 1:
                    fillers += p3_gens(sbi - 1)
                    n_units += 24
                attn(sbi, fillers, n_units)
            run_all(p3_gens(NSB - 1))

    nc.finalize()
    return nc


_NC = None


def _get_nc():
    global _NC
    if _NC is None:
        _NC = _build()
    return _NC


def kernel(x, wq, wk, wv, wo, wo_b):
    global LAST_RESULT
    x = np.ascontiguousarray(np.asarray(x, dtype=np.float32))
    wq = np.asarray(wq, dtype=np.float32)
    wk = np.asarray(wk, dtype=np.float32)
    wv = np.asarray(wv, dtype=np.float32)
    wo = np.asarray(wo, dtype=np.float32)
    wo_b = np.asarray(wo_b, dtype=np.float32)

    pp, ff = np.ogrid[0:128, 0:128]
    masks = (pp <= ff).astype(np.float32)

    in_maps = []
    for c in range(NCORES):
        b, hh = c // 2, c % 2
        es = slice(hh * EH, (hh + 1) * EH)
        in_maps.append(
            {
                "xt": np.ascontiguousarray(x[b].T.astype(MMNP)),
                "wqt": np.ascontiguousarray(wq[es, :].T.astype(MMNP)),
                "wkt": np.ascontiguousarray(wk[es, :].T.astype(MMNP)),
                "wvt": np.ascontiguousarray(wv[es, :].T.astype(MMNP)),
                "wot": np.ascontiguousarray(
                    wo[:, es].T.astype(MMNP)
                    .reshape(4, 2, 64, D)[:, ::-1]
                    .reshape(EH, D)
                ),
                "masks": masks.astype(MMNP),
            }
        )

    nc = _get_nc()
    res = run_bass_kernel_spmd(nc, in_maps, list(range(NCORES)), trace=TRACE)
    LAST_RESULT = res

    out = np.empty((B, S, D), np.float32)
    for b in range(B):
        out[b] = res.results[2 * b]["out"] + res.results[2 * b + 1]["out"]
    out += wo_b[None, None, :]
    return out
